# revision 26
# baseline (speedup 1.0000x reference)
"""Trainium2 Bass kernel for the differentiable gaussian renderer (v3).

Math per batch b, pixel (y,x), channel c:
    out[b,c,y,x] = num/den,  num = sum_n colors[n,c]*w[n,y,x],
    den = eps + sum_n w[n,y,x],
    w[n,y,x] = opac_n * exp(-((y-v_n)^2 + (x-u_n)^2)/(2 s_n^2))

Pixel kernel is separable: w = wy[n,y]*wx[n,x].  The host projects the
gaussians and ships, per core:
  - quadratic coefficients [A^2, 2A*By, By^2 - ln(opac), 2A*Bx, Bx^2] so the
    PE produces y/x arg^2 (with opacity folded into the y side) as k=5
    f32r matmuls against a [g^2, g, 1] basis, 256-wide into PSUM,
  - linear coefficients (A=rt/s, By, Bx) for DVE/Pool argbuild+square of the
    remaining chunks (opacity stays in the rhs via s4 for those),
  - s4 = [opac, opac*c] bf16 for sbuf chunks, c3 = colors bf16 for PE chunks.
exp runs on ACT in large groups (y-major for the matmul lhsT, x written
transposed for the 2x-mode rhs build), weights accumulate as bf16 matmuls
  psum[y,(k,x)] += Yexp[n,y] * rhs[n,(k,x)].
Tile-scheduler order is pinned with tile_wait_until logical times.

Sharding: 8 cores = 2 batches x 4 pixel-column strips of 32. No collectives.
"""

import math
from contextlib import ExitStack

import numpy as np

import concourse.bacc as bacc
import concourse.bass as bass
import concourse.mybir as mybir
import concourse.tile as tile
from concourse.bass_utils import run_bass_kernel_spmd

f32 = mybir.dt.float32
f32r = mybir.dt.float32r
bf16 = mybir.dt.bfloat16

H, W = 128, 128
FX, FY = 150.0, 150.0
CX, CY = 64.0, 64.0
EPS = 1e-8
N, B = 4096, 2
P = 128
J = N // P        # 32 chunks of 128 gaussians
NSTRIP = 4
SW = W // NSTRIP  # 32 columns per strip
RT = math.sqrt(0.5)
GRIDW = 256       # padded PE-argsq output width (>=256 unlocks 1 cyc/row f32r)

# ---- engine assignment knobs -------------------------------------------------
PE_DT = f32r        # argsq matmul dtype (f32 fallback if f32r misbehaves on HW)
E = 14              # PE-produced chunks [0, E) in two psum tiles (6+8)
EA = 6              # chunks [0, EA) in psqA, [EA, E) in psqB
DVE_RANGE = (14, 26)
POOL_RANGE = (26, 32)
N_WARM_MM = 8       # PE pstate-ramp warmup matmuls
# argbuild emission: (time, engine, j, axes) -- axes "yx", "y", or "x"
ARGB = ([(2.95 + 0.2 * i, "dve", j, "yx") for i, j in enumerate(range(*DVE_RANGE))]
        + [(2.95 + 0.42 * i, "pool", j, "yx")
           for i, j in enumerate(range(*POOL_RANGE))])
# square ops: (time, engine, a, b, axes) over sbuf-argbuilt chunks
SQ_OPS = [(3.3, "dve", 14, 16, "yx"), (4.6, "dve", 16, 21, "yx"),
          (5.9, "dve", 21, 26, "yx"), (5.3, "pool", 26, 32, "yx")]
# exp groups: (time, kind, a, b) with kind "psq" (slice of the PE tile)
# or "sbuf"; emission order = ACT program order
EXPY_OPS = [(3.55, "sbuf", 14, 16), (4.4, "psq", 0, 6), (5.3, "psq", 6, 14),
            (6.0, "sbuf", 16, 21), (6.9, "sbuf", 21, 26),
            (7.3, "sbuf", 26, 32)]
EXPX_OPS = [(3.6, "sbuf", 14, 16), (5.9, "psq", 0, 6), (6.1, "psq", 6, 14),
            (7.15, "sbuf", 16, 31), (7.95, "sbuf", 31, 32)]
# rhs groups: (time, a, b); chunks < E use the 3-block color rhs + direct
# den matmul, sbuf chunks use the 4-block s4 rhs
RHS_GROUPS = [(6.2, 14, 16), (6.45, 0, 6), (6.7, 6, 14),
              (7.45, 16, 26), (7.8, 26, 31), (8.1, 31, 32)]
T_DIV = 8.3
USE_KV_STORE = False  # store output via prepared kv_writeback + trigger

AF = mybir.ActivationFunctionType
ALU = mybir.AluOpType


def _emit(nc, tc, aps):
    with ExitStack() as ctx:
        pool = ctx.enter_context(tc.tile_pool(name="main", bufs=1))
        rhs_pool = ctx.enter_context(tc.tile_pool(name="rhs", bufs=3))
        psum_pool = ctx.enter_context(tc.tile_pool(name="psum", bufs=1, space="PSUM"))

        # ---------- input DMAs (issued immediately) ----------
        coef = pool.tile([5, J * P + GRIDW], f32r, tag="coef")
        pg = pool.tile([P, 210], f32, tag="pg")
        with tc.tile_wait_until(0.01):
            nc.sync.dma_start(pg[:], aps["pg"])          # SP/HWDGE: needed first
            nc.gpsimd.dma_start(coef[:], aps["coef"])    # Pool/SWDGE queue

        # ---------- constants + ACT table warm (in the DMA shadow) ----------
        warm = pool.tile([P, 1], f32, tag="warm")
        ones1 = pool.tile([1, P], f32, tag="ones1")
        epsr = pool.tile([1, SW], f32, tag="epsr")
        grid = pool.tile([P, 160], f32, tag="grid")
        with tc.tile_wait_until(0.1):
            nc.gpsimd.memset(warm[:], 0.0)
            nc.scalar.activation(warm[:], warm[:], AF.Exp)
            nc.gpsimd.memset(ones1[:], 1.0)
            nc.gpsimd.memset(epsr[:], EPS)
        with tc.tile_wait_until(0.3):
            # pixel-index grid [y(128) | x(32)] built on Pool (no DMA needed)
            nc.gpsimd.iota(grid[:], [[1, 160]], channel_multiplier=0,
                           allow_small_or_imprecise_dtypes=True)
            # x-part: (iota*1.0) + (strip*SW - 128), per-core values in pg
            nc.gpsimd.tensor_scalar(grid[:, 128:160], grid[:, 128:160],
                                    pg[:, 161:162], pg[:, 160:161],
                                    ALU.mult, ALU.add)

        # ---------- working tiles ----------
        argsY = pool.tile([P, J, H], f32, tag="argsY")
        argsXT = pool.tile([P, SW, J], f32, tag="argsXT")
        argsqY = pool.tile([P, J, H], f32, tag="argsqY")
        argsqXT = pool.tile([P, SW, J], f32, tag="argsqXT")
        expvY = pool.tile([P, J, H], bf16, tag="expvY")
        xeT = pool.tile([P, SW, J], bf16, tag="xeT")
        acc = psum_pool.tile([P, 4 * SW], f32, tag="acc")
        psqA = psum_pool.tile([P, EA, GRIDW], f32, tag="psqA")
        psqB = psum_pool.tile([P, E - EA, GRIDW], f32, tag="psqB")

        def psq_slice(a, b):
            """(tile, local a..b) for a chunk range within one psq tile."""
            if b <= EA:
                return psqA, a, b
            assert a >= EA
            return psqB, a - EA, b - EA
        s4 = pg[:, 96:160].bitcast(bf16)   # [P, 128] bf16: (k,j) at k*32+j
        c3 = pg[:, 162:210].bitcast(bf16)  # [P, 96] bf16: (k,j) at k*32+j, k<3
        s4p = s4.ap[0]
        c3p = c3.ap[0]
        gbase = J * P

        def emit_pe_argsq(j0, j1):
            gridr = coef[:, gbase:gbase + GRIDW]
            for j in range(j0, j1):
                t, a, _ = psq_slice(j, j + 1 if j + 1 <= EA else max(j + 1, EA + 1))
                t = psqA if j < EA else psqB
                i = j if j < EA else j - EA
                nc.tensor.matmul(t[:, i, :], coef[:, j * P:(j + 1) * P],
                                 gridr, start=True, stop=True)

        def emit_argbuild(eng, j, axes="yx"):
            e = nc.vector if eng == "dve" else nc.gpsimd
            if "y" in axes:
                e.tensor_scalar(argsY[:, j, :], grid[:, 0:H], pg[:, j:j + 1],
                                pg[:, 32 + j:33 + j], ALU.mult, ALU.add)
            if "x" in axes:
                e.tensor_scalar(argsXT[:, :, j], grid[:, H:160], pg[:, j:j + 1],
                                pg[:, 64 + j:65 + j], ALU.mult, ALU.add)

        def emit_sq(eng, a, b, axes="yx"):
            e = {"dve": nc.vector, "pool": nc.gpsimd}[eng]
            if "y" in axes and b > a:
                e.tensor_tensor(argsqY[:, a:b, :], argsY[:, a:b, :],
                                argsY[:, a:b, :], ALU.mult)
            if "x" in axes and b > a:
                e.tensor_tensor(argsqXT[:, :, a:b], argsXT[:, :, a:b],
                                argsXT[:, :, a:b], ALU.mult)

        def emit_expy(kind, a, b):
            if kind == "psq":
                t, la, lb = psq_slice(a, b)
                nc.scalar.activation(expvY[:, a:b, :], t[:, la:lb, 0:H],
                                     AF.Exp, scale=-1.0)
            else:
                nc.scalar.activation(expvY[:, a:b, :], argsqY[:, a:b, :],
                                     AF.Exp, scale=-1.0)

        def emit_expx(kind, a, b):
            if kind == "psq":
                t, la, lb = psq_slice(a, b)
                n = b - a
                src = bass.AP(t[:].tensor, t[:, la, H].offset,
                              [t[:].ap[0], [GRIDW, n], [1, SW]])
                dst = bass.AP(xeT[:].tensor, xeT[:, 0, a].offset,
                              [xeT[:].ap[0], [1, n], [J, SW]])
                nc.scalar.activation(dst, src, AF.Exp, scale=-1.0)
            else:
                nc.scalar.activation(xeT[:, :, a:b], argsqXT[:, :, a:b],
                                     AF.Exp, scale=-1.0)

        def emit_rhs_acc(a, b):
            n = b - a
            if b <= E:  # PE chunks: 3-block color rhs + direct den matmul
                rhs = rhs_pool.tile([P, 3, SW, n], bf16, tag="rhs3")
                xrep = bass.AP(xeT[:].tensor, xeT[:, 0, a].offset,
                               [xeT[:].ap[0], [0, 3], [J, SW], [1, n]])
                srep = bass.AP(c3.tensor, c3.offset + a,
                               [c3p, [32, 3], [0, SW], [1, n]])
                nc.vector.tensor_tensor(rhs[:], xrep, srep, ALU.mult)
                for j in range(a, b):
                    nc.tensor.matmul(acc[:, 0:SW], expvY[:, j, :],
                                     xeT[:, :, j], start=False, stop=False)
                    rslice = bass.AP(rhs[:].tensor, rhs[:].offset + (j - a),
                                     [rhs[:].ap[0], [SW * n, 3], [n, SW]])
                    nc.tensor.matmul(acc[:, SW:4 * SW], expvY[:, j, :], rslice,
                                     start=False, stop=False)
            else:       # sbuf chunks: 4-block s4 rhs
                rhs = rhs_pool.tile([P, 4, SW, n], bf16, tag="rhs4")
                xrep = bass.AP(xeT[:].tensor, xeT[:, 0, a].offset,
                               [xeT[:].ap[0], [0, 4], [J, SW], [1, n]])
                srep = bass.AP(s4.tensor, s4.offset + a,
                               [s4p, [32, 4], [0, SW], [1, n]])
                nc.vector.tensor_tensor(rhs[:], xrep, srep, ALU.mult)
                for j in range(a, b):
                    rslice = bass.AP(rhs[:].tensor, rhs[:].offset + (j - a),
                                     [rhs[:].ap[0], [SW * n, 4], [n, SW]])
                    nc.tensor.matmul(acc[:], expvY[:, j, :], rslice,
                                     start=False, stop=(j == J - 1))

        # ---------- emission schedule (logical times pin the scheduler) -----
        with tc.tile_wait_until(0.5):
            # PE pstate-ramp warmup into the acc bank, then the eps preload
            # opens the real accumulation group (removes eps from the tail)
            for _ in range(N_WARM_MM):
                nc.tensor.matmul(acc[:, 0:SW], ones1[:], epsr[:],
                                 start=True, stop=True)
            nc.tensor.matmul(acc[:, 0:SW], ones1[:], epsr[:],
                             start=True, stop=False)

        with tc.tile_wait_until(3.2):
            emit_pe_argsq(0, E)
        for t, eng, j, axes in ARGB:
            with tc.tile_wait_until(t):
                emit_argbuild(eng, j, axes)
        for t, eng, a, b, axes in SQ_OPS:
            with tc.tile_wait_until(t):
                emit_sq(eng, a, b, axes)
        for t, kind, a, b in EXPY_OPS:
            with tc.tile_wait_until(t):
                emit_expy(kind, a, b)
        for t, kind, a, b in EXPX_OPS:
            with tc.tile_wait_until(t):
                emit_expx(kind, a, b)
        for t, a, b in RHS_GROUPS:
            with tc.tile_wait_until(t):
                emit_rhs_acc(a, b)

        # ---------- divide and store ----------
        outsb = pool.tile([P, 3, SW], bf16, tag="outsb")
        if USE_KV_STORE:
            # pre-generate the store descriptors early on the idle Pool
            # engine; the trigger (cheap) fires once outsb is written
            idx0 = pool.tile([P, 1], mybir.dt.int32, tag="idx0")
            dma_sem = nc.alloc_semaphore("out_dma_sem")
            with tc.tile_wait_until(1.0):
                nc.gpsimd.memset(idx0[:], 0.0)
                ot = aps["out"].tensor
                out4 = bass.AP(ot, 0, [[0, 1], [3 * SW, P], [SW, 3], [1, SW]])
                in4 = bass.AP(outsb[:].tensor, outsb[:].offset,
                              [outsb[:].ap[0], [SW, 3], [SW, 1], [1, SW]])
                nc.gpsimd.kv_writeback(out4, in4, idx0[:],
                                       prepare_only=True, sem=dma_sem)
        with tc.tile_wait_until(T_DIV):
            dinv = pool.tile([P, SW], f32, tag="dinv")
            nc.vector.reciprocal_approx_fast(dinv[:], acc[:, 0:SW])
            dinv3 = bass.AP(dinv[:].tensor, dinv[:].offset,
                            [dinv[:].ap[0], [0, 3], [1, SW]])
            nc.vector.tensor_tensor(outsb[:], acc[:, SW:4 * SW], dinv3, ALU.mult)
            if not USE_KV_STORE:
                nc.sync.dma_start(aps["out"].rearrange("c y x -> y c x"),
                                  outsb[:])
        if USE_KV_STORE:
            with tc.tile_wait_until(T_DIV + 0.2):
                nc.gpsimd.trigger_dma(count=None)
            with tc.tile_wait_until(T_DIV + 0.4):
                nc.gpsimd.wait_ge(dma_sem, 16)


def build_nc(num_devices=8, debug=False):
    nc = bacc.Bacc(
        "TRN2", target_bir_lowering=False, debug=debug, num_devices=num_devices
    )
    aps = {
        "coef": nc.dram_tensor("coef", [5, J * P + GRIDW], f32r,
                               kind="ExternalInput").ap(),
        "pg": nc.dram_tensor("pg", [P, 210], f32, kind="ExternalInput").ap(),
        "out": nc.dram_tensor("out", [H, 3, SW] if USE_KV_STORE else [3, H, SW],
                              bf16, kind="ExternalOutput").ap(),
    }
    with tile.TileContext(nc) as tc:
        _emit(nc, tc, aps)
    nc.compile()
    return nc


def _quat_to_rot(q):
    q = q / np.linalg.norm(q)
    w, x, y, z = q
    return np.array([
        [1 - 2 * (y * y + z * z), 2 * (x * y - z * w), 2 * (x * z + y * w)],
        [2 * (x * y + z * w), 1 - 2 * (x * x + z * z), 2 * (y * z - x * w)],
        [2 * (x * z - y * w), 2 * (y * z + x * w), 1 - 2 * (x * x + y * y)],
    ], dtype=np.float64)


def _bf16_bits(x):
    """Round f32 array to bf16, return as packed uint16."""
    u = np.asarray(x, np.float32).view(np.uint32)
    r = ((u >> 16) + ((u >> 15) & 1)).astype(np.uint32)
    return (r & 0xFFFF).astype(np.uint16)


def _pack_bf16_pairs(bits):
    """[P, 2n] uint16 -> [P, n] f32 with little-endian bf16 pairs."""
    return (bits[:, 0::2].astype(np.uint32)
            | (bits[:, 1::2].astype(np.uint32) << 16)).view(np.float32)


def make_in_maps(positions, colors, opacities, scales, qvec, tvec):
    positions = np.asarray(positions, np.float64)
    colors = np.asarray(colors, np.float32)
    opacities = np.asarray(opacities, np.float32).reshape(N)
    scales = np.asarray(scales, np.float64).reshape(N)
    qvec = np.asarray(qvec, np.float64)
    tvec = np.asarray(tvec, np.float64)
    lnop = np.log(np.maximum(opacities.astype(np.float64), 1e-300))

    in_maps = []
    for core in range(8):
        b, strip = core // NSTRIP, core % NSTRIP
        R = _quat_to_rot(qvec[b])
        pc = positions @ R.T + tvec[b]
        with np.errstate(divide="ignore", invalid="ignore"):
            u = pc[:, 0] / pc[:, 2] * FX + CX
            v = pc[:, 1] / pc[:, 2] * FY + CY
        u = np.clip(np.nan_to_num(u, nan=1e6, posinf=1e6, neginf=-1e6), -1e6, 1e6)
        v = np.clip(np.nan_to_num(v, nan=1e6, posinf=1e6, neginf=-1e6), -1e6, 1e6)
        # The PE argsq path (chunks < E) runs at reduced f32r precision; its
        # error is only visible for gaussians whose footprint reaches the
        # screen.  Permute so near-screen gaussians fill the full-precision
        # DVE/Pool chunks (tail indices) first; far ones go to the PE chunks.
        # (The weight sum is permutation-invariant.)
        near = ((np.abs(u - CX) < 400) & (np.abs(v - CY) < 400)).astype(np.int64)
        order = np.argsort(near, kind="stable")  # far first -> PE chunks
        uo, vo = u[order], v[order]
        so = scales[order]
        opo = opacities[order]
        lno = lnop[order]
        co = colors[order]
        A = RT / so
        By = -vo * A
        Bx = -uo * A

        # coef[5, J*P + GRIDW]: per-chunk quadratic rows + grid basis.
        # Chunks < E have ln(opac) folded into the y-side constant term.
        c2 = By * By
        c2[:E * P] -= lno[:E * P]
        coef = np.zeros((5, J * P + GRIDW), np.float32)
        cg = np.stack([A * A, 2 * A * By, c2, 2 * A * Bx, Bx * Bx])
        coef[:, 0:J * P] = cg.reshape(5, J, P).astype(np.float32).reshape(5, J * P)
        g = J * P
        yg = np.arange(H, dtype=np.float64)
        xg = np.arange(SW, dtype=np.float64) + strip * SW
        coef[0, g:g + H] = (yg * yg).astype(np.float32)
        coef[0, g + H:g + H + SW] = (xg * xg).astype(np.float32)
        coef[1, g:g + H] = yg.astype(np.float32)
        coef[2, g:g + H] = 1.0
        coef[3, g + H:g + H + SW] = xg.astype(np.float32)
        coef[4, g + H:g + H + SW] = 1.0

        # pg[128, 210]: A | By | Bx | s4(bf16) | xoff | 1.0 | c3(bf16)
        pg = np.zeros((P, 210), np.float32)
        pg[:, 0:32] = A.reshape(J, P).T.astype(np.float32)
        pg[:, 32:64] = By.reshape(J, P).T.astype(np.float32)
        pg[:, 64:96] = Bx.reshape(J, P).T.astype(np.float32)
        s4 = np.empty((P, 4, J), np.float32)
        s4[:, 0, :] = opo.reshape(J, P).T
        for c in range(3):
            s4[:, 1 + c, :] = (opo * co[:, c]).reshape(J, P).T
        pg[:, 96:160] = _pack_bf16_pairs(_bf16_bits(s4.reshape(P, 128)))
        pg[:, 160] = float(strip * SW - 128)
        pg[:, 161] = 1.0
        c3a = np.zeros((P, 3, J), np.float32)
        for c in range(3):
            c3a[:, c, :] = co[:, c].reshape(J, P).T
        pg[:, 162:210] = _pack_bf16_pairs(_bf16_bits(c3a.reshape(P, 96)))
        in_maps.append({"coef": coef, "pg": pg})
    return in_maps


_NC_CACHE = {}


def _get_nc():
    if "nc" not in _NC_CACHE:
        _NC_CACHE["nc"] = build_nc()
    return _NC_CACHE["nc"]


def run_spmd(inputs, trace=False, **kw):
    nc = _get_nc()
    in_maps = make_in_maps(**inputs)
    try:
        res = run_bass_kernel_spmd(nc, in_maps, list(range(8)), trace=trace, **kw)
    except Exception:
        res = run_bass_kernel_spmd(nc, in_maps, list(range(8)), trace=trace, **kw)
    out = np.empty((B, 3, H, W), np.float32)
    for core in range(8):
        b, strip = core // NSTRIP, core % NSTRIP
        o = np.asarray(res.results[core]["out"], np.float32)
        if USE_KV_STORE:
            o = o.transpose(1, 0, 2)
        out[b, :, :, strip * SW:(strip + 1) * SW] = o
    return out, res


def kernel(positions, colors, opacities, scales, qvec, tvec):
    out, _ = run_spmd(dict(
        positions=positions, colors=colors, opacities=opacities,
        scales=scales, qvec=qvec, tvec=tvec,
    ))
    return out
